# revision 4
# baseline (speedup 1.0000x reference)
"""Trainium2 Bass kernel for nn_Dyanmic_Q_MLP (fake-quant MLP) — v2.

Computation (reference):
    w1q = fake_quant(w1, 8); w2q = fake_quant(w2, 8)       # per-tensor symmetric
    h   = relu(x @ w1q.T + b1)                             # [B,S,3072]
    out = h @ w2q.T + b2                                   # [B,S,768]

Cost-model time ~240 us/core (v1 baseline: 434 us); measured HW rel err
4.1e-3 vs the 2e-2 gate.  Design:
  * Data-parallel over the flattened (B*S)=12544 rows across 8 cores
    (1568 rows/core, 7 blocks of 224).  Weights replicated, no
    collectives (an AllReduce costs a flat ~28us in this fabric, more
    than the scan it could shard).
  * SINGLE-pass bf16 matmuls: int8-quantized weights are EXACT in bf16,
    x/h in bf16 contribute only ~2.4e-3, so the v1 hi/lo split (2x PE
    work for fp32 accuracy) is dropped.  PE busy ~191us = the bf16
    streaming floor for 2 x [1568,768]x[768,3072] per core.
  * Each weight tensor is DMA'd exactly ONCE (v1: twice).  The abs-max
    scan stages an fp16 copy directly into the w1q/w2q tiles (through a
    bitcast view), and pass 2 quantizes IN PLACE from fp16:
    round(fp16(w)*inv_s) via the +-1.5*2^23 RNE-add trick.  fp16's
    11-bit significand keeps the round-one-step-differently rate ~0.9%,
    worth ~4e-3 output error; bf16 staging (8 bits) would cost ~1.9e-2.
    Halved DMA lets w2's scan finish by ~65us so fc2 never starves.
  * All abs-max reduces accumulate on DVE.  The Pool (gpsimd) XYZWC
    reduce SILENTLY IGNORES apply_absolute_value on HW - computing s2
    from a signed max shifts every rounding boundary and decorrelates
    q from the reference grid (~1.7e-2 rel err).  Pool instead runs a
    1/3 share of the in-place quant streams (tensor_scalar is exact).
  * Emission is a single interleaved schedule: per hidden t-group tau,
    [w1 in-place slice (data resident)] -> [fc1 t-group for the first
    WAVE=3 row blocks] -> [w2 scan step (DMA-paced; last so it cannot
    head-of-line block the epilogues in the in-order ACT/DVE queues)].
    fc1 runs FC1_AHEAD=4 blocks before fc2(0); fc2(0) accumulates t in
    w2q production-completion order (sums commute).
  * Fused fc1 epilogue (one ACT op: relu(psum + b1/s1) -> bf16); the
    final scale s1*s2 and +b2 fold into fc2's single ACT epilogue.
  * x / out move as one descriptor-batch per row block via [128, KD, M]
    DRAM layouts (host does layout-only transposes); consts ride the
    ACT HWDGE queue so the scan owns SP+generator from t=0; block 0's
    x is gated one chunk before scan end to land right as it drains.
"""

import sys

for _p in ("/opt/trn_rl_repo", "/root/.axon_site/_ro/trn_rl_repo"):
    if _p not in sys.path:
        sys.path.insert(0, _p)

from contextlib import ExitStack

import numpy as np

import concourse.bass as bass
import concourse.mybir as mybir
import concourse.tile as tile
from concourse import bass_utils
from concourse.tile_rust import add_dep_helper

N_CORES = 8
B, S, D, H = 64, 196, 768, 3072
M_TOTAL = B * S            # 12544
M_SHARD = M_TOTAL // N_CORES   # 1568
M_PAD = M_SHARD
M_BLOCKS = [224] * 7
WAVE = 3                   # leading fc1 blocks emitted t-group-major
FC1_AHEAD = 4              # fc1 blocks emitted before the first fc2 block
KD = D // 128              # 6
KH = H // 128              # 24
C_RNE = 12582912.0         # 1.5 * 2**23: (v + C) - C == round-to-nearest-even(v)

F32 = mybir.dt.float32
BF16 = mybir.dt.bfloat16
F16 = mybir.dt.float16
ALU = mybir.AluOpType
ACTF = mybir.ActivationFunctionType


def _split_oversized_waits(nc, max_waits=1):
    """The walrus build in this container accepts only one sync-wait per
    instruction.  Hoist excess on_wait entries onto inserted same-engine
    NoOp instructions placed just before."""
    for f in nc.m.functions:
        for b in f.blocks:
            new_list, changed, ctr = [], False, 0
            for i in b.instructions:
                si = i.sync_info
                w = list(si.on_wait) if si is not None else []
                if len(w) > max_waits:
                    extra, keep = w[:-max_waits], w[-max_waits:]
                    for ci in range(0, len(extra), max_waits):
                        ctr += 1
                        d = mybir.InstNoOp(
                            name=f"{i.name}-wsplit{ctr}",
                            engine=i.engine,
                        )
                        d.sync_info = mybir.SyncInfo(
                            on_update=[], on_wait=extra[ci : ci + max_waits]
                        )
                        new_list.append(d)
                    si.on_wait = keep
                    changed = True
                new_list.append(i)
            if changed:
                b.instructions = new_list


def build_program(qmax: float, walrus_fixups: bool = True):
    nc = bass.Bass("TRN2", target_bir_lowering=False, debug=False)

    xt_d = nc.dram_tensor("xt", (128, KD, M_PAD), F32, kind="ExternalInput").ap()
    w1t_d = nc.dram_tensor("w1t", (D, H), F32, kind="ExternalInput").ap()
    w2t_d = nc.dram_tensor("w2t", (H, D), F32, kind="ExternalInput").ap()
    b1_d = nc.dram_tensor("b1", (128, KH), F32, kind="ExternalInput").ap()
    b2_d = nc.dram_tensor("b2", (128, KD), F32, kind="ExternalInput").ap()
    id_d = nc.dram_tensor("ident", (128, 128), F32, kind="ExternalInput").ap()
    out_d = nc.dram_tensor("outT", (128, KD, M_PAD), F32, kind="ExternalOutput").ap()

    with tile.TileContext(nc) as tc, ExitStack() as ctx:
        const = ctx.enter_context(tc.tile_pool(name="const", bufs=1))
        wq = ctx.enter_context(tc.tile_pool(name="wq", bufs=1))
        wstage = ctx.enter_context(tc.tile_pool(name="wstage", bufs=2))
        xstage = ctx.enter_context(tc.tile_pool(name="xstage", bufs=3))
        xhp = ctx.enter_context(tc.tile_pool(name="xhp", bufs=WAVE + 1))
        hp = ctx.enter_context(tc.tile_pool(name="hp", bufs=FC1_AHEAD + 1))
        opool = ctx.enter_context(tc.tile_pool(name="opool", bufs=1))
        scal = ctx.enter_context(tc.tile_pool(name="scal", bufs=1))
        ps1 = ctx.enter_context(tc.tile_pool(name="ps1", bufs=4, space="PSUM"))
        ps2 = ctx.enter_context(tc.tile_pool(name="ps2", bufs=3, space="PSUM"))

        # ---------- setup (const DMAs on the ACT HWDGE queue so the w1
        # scan owns the SP queue + generator from t=0) ----------
        b1_pack = const.tile([128, KH], F32, tag="b1pack")
        nc.scalar.dma_start(b1_pack[:], b1_d[:])
        b2_pack = const.tile([128, KD], F32, tag="b2pack")
        nc.scalar.dma_start(b2_pack[:], b2_d[:])
        ident = const.tile([128, 128], F32, tag="ident")
        nc.scalar.dma_start(ident[:], id_d[:])
        ones_row = const.tile([1, 128], F32, tag="ones_row")
        nc.vector.memset(ones_row[:], 1.0)
        cpos = const.tile([128, 1], F32, tag="cpos")
        nc.vector.memset(cpos[:], C_RNE)
        cneg = const.tile([128, 1], F32, tag="cneg")
        nc.vector.memset(cneg[:], -C_RNE)

        # ---------- fake-quant: pass 1 (abs-max scan) ----------
        def quant_pass1_gen(wt_d, n_rows, free_dim, chunk, tag, result,
                            gate_dma_on=None, p1_bufs=5, on_pool=False,
                            stage_to=None, stage_engines=(None,)):
            """Generator: yields after emitting each chunk's DMA+reduce so
            callers can interleave other DMA streams on the SP queue.
            Streams wt_d, computes global abs-max, replicates it to every
            partition via exact PE f32 transposes.  Fills `result` with
            scale [128,1], macc_last, and inv_fn (emits the DVE reciprocal
            when called, so its DVE queue slot is caller-controlled).

            on_pool: run reduces + chain on Pool (gpsimd), which only
            supports full XYZWC reduces -> accumulate a [1,1] max."""
            n_chunks = free_dim // chunk
            total = n_rows * n_chunks
            # Pool (gpsimd) supports reduce but NOT TensorTensor(max) on HW:
            # each chunk's XYZWC reduce writes one column of mk_all, and a
            # single final reduce collapses it -- no accumulate chain.
            eng = nc.gpsimd if on_pool else nc.vector
            if on_pool:
                mk_all = scal.tile([1, total], F32, tag=f"{tag}mkall",
                                   name=f"{tag}mkall")
            else:
                macc = scal.tile([128, 1], F32, tag=f"{tag}macc",
                                 name=f"{tag}macc")
            first = True
            macc_last = None
            for k in range(n_rows):
                for j in range(n_chunks):
                    ci = k * n_chunks + j
                    wst = wstage.tile([128, chunk], F32, tag=f"{tag}st",
                                      name=f"{tag}st", bufs=p1_bufs)
                    dma = nc.sync.dma_start(
                        wst[:], wt_d[k * 128:(k + 1) * 128, j * chunk:(j + 1) * chunk]
                    )
                    if gate_dma_on is not None:
                        add_dep_helper(dma.ins, gate_dma_on,
                                       reason="serialize bulk weight DMA streams")
                    if on_pool:
                        macc_last = eng.tensor_reduce(
                            mk_all[:, ci:ci + 1], wst[:],
                            axis=mybir.AxisListType.XYZWC,
                            op=ALU.max, apply_absolute_value=True,
                        )
                    else:
                        mk = scal.tile([128, 1], F32, tag=f"{tag}mk",
                                       name=f"{tag}mk", bufs=2)
                        eng.tensor_reduce(
                            mk[:], wst[:], axis=mybir.AxisListType.X,
                            op=ALU.max, apply_absolute_value=True,
                        )
                        if first:
                            macc_last = eng.tensor_copy(macc[:], mk[:])
                            first = False
                        else:
                            macc_last = eng.tensor_tensor(
                                macc[:], macc[:], mk[:], op=ALU.max
                            )
                    result["macc_last"] = macc_last
                    if ci == total - 2:
                        # handle for gating work to start right as the scan
                        # drains (one chunk before the end)
                        result["late_reduce"] = macc_last
                    if stage_to is not None:
                        # keep an fp16 copy resident (11-bit significand --
                        # ~4x less round-one-step-differently noise than
                        # bf16) so pass 2 needs no re-DMA.  Written through
                        # an fp16 bitcast view of the bf16 dst tile.
                        se = stage_engines[ci % len(stage_engines)]
                        dst = stage_to[k][:, j * chunk:(j + 1) * chunk]
                        dst = dst.bitcast(F16)
                        if se is nc.scalar:
                            nc.scalar.activation(dst, wst[:], ACTF.Copy)
                        else:
                            se.tensor_copy(dst, wst[:])
                    yield
            if on_pool:
                g11 = scal.tile([1, 1], F32, tag=f"{tag}g11", name=f"{tag}g11")
                macc_last = eng.tensor_reduce(
                    g11[:], mk_all[:], axis=mybir.AxisListType.XYZWC,
                    op=ALU.max,
                )
                result["macc_last"] = macc_last
                # the rest of the chain runs on DVE (fires late, when the
                # DVE queue has drained the w1 slice stream)
                eng = nc.vector
            else:
                # [128,1] -T-> [1,128](PSUM) -max-> [1,1]  (DVE reads PSUM
                # directly; no staging copy)
                rps = ps2.tile([1, 128], F32, tag="redT", name=f"{tag}rps", bufs=1)
                nc.tensor.transpose(rps[:], macc[:], ident[:])
                g11 = scal.tile([1, 1], F32, tag=f"{tag}g11", name=f"{tag}g11")
                eng.tensor_reduce(g11[:], rps[:], axis=mybir.AxisListType.X,
                                  op=ALU.max)
            # [1,1] -bcast-> [1,128] -T-> [128,1]
            grow = scal.tile([1, 128], F32, tag=f"{tag}grow", name=f"{tag}grow")
            eng.tensor_scalar(grow[:], ones_row[:], g11[:], None, op0=ALU.mult)
            gps = ps2.tile([128, 1], F32, tag="redT", name=f"{tag}gps", bufs=1)
            nc.tensor.transpose(gps[:], grow[:], ident[:1, :1])
            # mult by 1/qmax straight out of PSUM (DVE reads PSUM):
            # <=1 ulp from the reference max/qmax
            scale = scal.tile([128, 1], F32, tag=f"{tag}scale", name=f"{tag}scale")
            nc.vector.tensor_scalar(scale[:], gps[:], 1.0 / float(qmax), None,
                                    op0=ALU.mult)

            def inv_fn():
                inv_s = scal.tile([128, 1], F32, tag=f"{tag}inv", name=f"{tag}inv")
                nc.vector.reciprocal(inv_s[:], scale[:])
                return inv_s

            result["scale"] = scale
            result["inv_fn"] = inv_fn

        def quant_pass1(wt_d, n_rows, free_dim, chunk, tag, **kw):
            r = {}
            for _ in quant_pass1_gen(wt_d, n_rows, free_dim, chunk, tag, r, **kw):
                pass
            return r["scale"], r["inv_fn"](), r["macc_last"]

        # ---------- fake-quant: pass 2 (in-place round(w_bf16*inv_s)) -------
        def inplace_quant(dst, e, tag, cw):
            """Quantize a bf16-staged slice in place: dst = (dst*inv_s + C)
            - C via an f32 temp (the RNE add trick needs f32 precision).
            Reads the bf16 staging, writes exact-int bf16.  Temp rotors are
            per-engine so one engine's backlog can't stall another's."""
            sfx = {id(nc.vector): "v", id(nc.scalar): "a",
                   id(nc.gpsimd): "p"}[id(e)]
            tmp = wstage.tile([128, cw], F32, tag=f"{tag}tmp{sfx}",
                              name=f"{tag}tmp", bufs=1)
            src = dst.bitcast(F16)   # the staging wrote fp16 bits
            if e is nc.scalar:
                nc.scalar.activation(tmp[:, :cw], src, ACTF.Identity,
                                     bias=cpos[:], scale=inplace_quant.inv[tag][:])
                nc.scalar.activation(dst, tmp[:, :cw], ACTF.Identity,
                                     bias=cneg[:])
            else:
                e.tensor_scalar(tmp[:, :cw], src, inplace_quant.inv[tag][:], C_RNE,
                                op0=ALU.mult, op1=ALU.add)
                e.tensor_scalar(dst, tmp[:, :cw], C_RNE, None, op0=ALU.subtract)
        inplace_quant.inv = {}

        w1q = [wq.tile([128, H], BF16, tag=f"w1q{d}", name=f"w1q{d}")
               for d in range(KD)]
        w2q = [wq.tile([128, D], BF16, tag=f"w2q{t}", name=f"w2q{t}")
               for t in range(KH)]

        # ---------- x load + bf16 conversion (one DMA + one conv per block) --
        def load_x_block(m0, m_blk, gate_on=None, conv_eng=nc.scalar):
            xsb = xstage.tile([128, KD, m_blk], F32, tag="xsb", name="xsb")
            xdma = nc.gpsimd.dma_start(xsb[:], xt_d[:, :, m0:m0 + m_blk])
            if gate_on is not None:
                add_dep_helper(xdma.ins, gate_on,
                               reason="x block after w1 pass1 scan")
            xhb = xhp.tile([128, KD, m_blk], BF16, tag="xhb", name="xhb")
            if conv_eng is nc.scalar:
                nc.scalar.activation(xhb[:], xsb[:], ACTF.Copy)
            else:
                conv_eng.tensor_copy(xhb[:], xsb[:])
            return xhb

        # ---------- main pipeline ----------
        # Both weight tensors are DMA'd exactly ONCE: the abs-max scan also
        # stages a bf16 copy directly into the w1q/w2q tiles, and pass 2 is
        # a pure in-place SBUF op stream round(bf16(w)*inv_s) with no
        # re-DMA.  Cost: ~2.7% of weight entries round one step differently
        # than the f32 reference (bf16 staging noise), ~1e-2 output rel err
        # vs the 2e-2 gate; benefit: weight DMA traffic halves, so the w2
        # scan finishes ~35us earlier and the PE never waits on the wire.
        V, A = nc.vector, nc.scalar
        r1 = {}
        for _ in quant_pass1_gen(w1t_d, KD, H, 768, "q1", r1,
                                 stage_to=w1q, stage_engines=(A,)):
            pass
        s1 = r1["scale"]
        q1_macc = r1["macc_last"]
        inv_s1 = r1["inv_fn"]()
        inplace_quant.inv["q1"] = inv_s1
        # b1' = b1 / s1 (emitted on DVE right behind the reciprocal)
        b1s = const.tile([128, KH], F32, tag="b1s")
        nc.vector.tensor_scalar(b1s[:], b1_pack[:], inv_s1[:], None, op0=ALU.mult)

        starts = []
        o = 0
        for mb in M_BLOCKS:
            starts.append(o)
            o += mb
        xh = {}
        # block 0's x rides right behind the last scan chunk so its
        # conversion is done before the PE needs it
        xh[0] = load_x_block(starts[0], M_BLOCKS[0],
                             gate_on=r1["late_reduce"].ins,
                             conv_eng=nc.scalar)
        # w1 in-place quant, fine-first: cols 0:128 as [128,128] slices
        # (split DVE/ACT) so the PE starts ~1us after the scan chain; the
        # rest emitted interleaved with the wave below.
        for k in range(KD):
            inplace_quant(w1q[k][:, 0:128], (V if k % 2 == 0 else A),
                          "q1", 128)
        w1_slices = [(k, 128, 640) for k in range(KD)]
        for j in range(1, H // 768):
            w1_slices += [(k, j * 768, 768) for k in range(KD)]
        # the other wave blocks' x (gated off the scan window); their
        # conversions go on DVE so the ACT queue reaches the j0-remainder
        # quant ops (queued behind them) without serializing on x DMAs
        for b in range(1, WAVE):
            xh[b] = load_x_block(starts[b], M_BLOCKS[b], gate_on=q1_macc.ins,
                                 conv_eng=nc.vector)
        # front-load the rest of j-window 0 (t-groups 1-5), split DVE/ACT
        for k in range(KD):
            inplace_quant(w1q[k][:, 128:768], (V if k % 2 == 0 else A),
                          "q1", 640)

        def fc1_tgroup(b, t, hh_list):
            m_blk = M_BLOCKS[b]
            ps = ps1.tile([128, m_blk], F32, tag="ps1", name="ps1")
            for d in range(KD):
                nc.tensor.matmul(
                    ps[:], w1q[d][:, t * 128:(t + 1) * 128], xh[b][:, d, :],
                    start=(d == 0), stop=(d == KD - 1),
                )
            hh_ = hp.tile([128, m_blk], BF16, tag=f"hh{t}", name=f"hh{t}")
            nc.scalar.activation(hh_[:], ps[:], ACTF.Relu, bias=b1s[:, t:t + 1])
            hh_list.append(hh_)

        # fc2 block 0 accumulates t in w2q production-completion order
        # (DVE evens, then Pool odds, then ACT late odds); sums commute.
        T_PROD_ORDER = ([t for t in range(KH) if t % 2 == 0]
                        + [t for t in range(KH) if t % 2 == 1 and t < 12]
                        + [t for t in range(KH) if t % 2 == 1 and t >= 12])

        def fc2_block(b, hh_list, split_out=False, t_order=None):
            m0, m_blk = starts[b], M_BLOCKS[b]
            t_order = t_order or list(range(KH))
            otb = opool.tile([128, KD, m_blk], F32, tag="otb", name="otb")
            for dt in range(KD):
                ps_ = ps2.tile([128, m_blk], F32, tag="ps2", name="ps2")
                for ti, t in enumerate(t_order):
                    nc.tensor.matmul(
                        ps_[:], w2q[t][:, dt * 128:(dt + 1) * 128], hh_list[t][:],
                        start=(ti == 0), stop=(ti == KH - 1),
                    )
                nc.scalar.activation(
                    otb[:, dt, :], ps_[:], ACTF.Identity,
                    bias=b2_pack[:, dt:dt + 1], scale=cscale[:],
                )
                if split_out and dt == KD - 2:
                    # overlap most of the final out-DMA with the last
                    # dt-group to shorten the tail
                    nc.sync.dma_start(out_d[:, :KD - 1, m0:m0 + m_blk],
                                      otb[:, :KD - 1, :])
            if split_out:
                nc.sync.dma_start(out_d[:, KD - 1:, m0:m0 + m_blk],
                                  otb[:, KD - 1:, :])
            else:
                nc.sync.dma_start(out_d[:, :, m0:m0 + m_blk], otb[:])

        hh = {b: [] for b in range(len(M_BLOCKS))}
        # Unified wave loop: per t-group tau emit (a) one w2 scan step
        # (DMA + Pool reduce + bf16 stage into w2q), (b) the w1 in-place
        # slice producing t-groups tau+6.., (c) the wave t-group for the
        # first WAVE blocks.  This keeps every engine's in-order queue
        # sorted by expected execution time.
        # w2's abs-max runs on DVE (X-reduce + accumulate): the Pool XYZWC
        # reduce SILENTLY IGNORES apply_absolute_value on HW, which skews
        # s2 by a few percent and decorrelates every rounding decision vs
        # the reference (~1.7e-2 rel err).  DVE absorbs it by handing 1/2
        # of the w1 slice stream to Pool (gpsimd tensor_scalar is exact).
        r2 = {}
        g2p1 = quant_pass1_gen(w2t_d, KH, D, 768, "q2", r2, on_pool=False,
                               gate_dma_on=q1_macc.ins, p1_bufs=5,
                               stage_to=w2q, stage_engines=(A, A, V))
        w1_rest = w1_slices[KD:]          # j-windows 1..3, 18 slices
        for tau in range(KH):
            # w1 slice BEFORE the scan step: its data is already resident,
            # so it must sit ahead of the DMA-paced reduce in the DVE queue
            if tau < len(w1_rest):
                k, c0, cw = w1_rest[tau]
                inplace_quant(w1q[k][:, c0:c0 + cw],
                              (nc.gpsimd if tau % 2 == 1 else V), "q1", cw)
            for b in range(WAVE):
                fc1_tgroup(b, tau, hh[b])
            # scan step LAST within each tau: its convert/reduce are paced
            # by the w2 DMA and must not head-of-line block the epilogues
            next(g2p1, None)
        for _ in g2p1:                     # drain (chain ops on Pool)
            pass
        inv_s2 = r2["inv_fn"]()
        inplace_quant.inv["q2"] = inv_s2
        s2 = r2["scale"]
        # c = s1 * s2 (final output scale), per-partition [128,1]
        cscale = scal.tile([128, 1], F32, tag="cscale")
        nc.vector.tensor_tensor(cscale[:], s1[:], s2[:], op=ALU.mult)

        # then FC1_AHEAD-WAVE more plain fc1 blocks before the first fc2;
        # the w2 in-place quant stream (DVE + Pool, t-interleaved) is
        # emitted before fc2 block 0 consumes it t-ascending.
        n_blocks = len(M_BLOCKS)
        fc1_left = list(range(WAVE, n_blocks))
        fc2_left = list(range(n_blocks))
        for _ in range(FC1_AHEAD - WAVE):
            b = fc1_left.pop(0)
            xh[b] = load_x_block(starts[b], M_BLOCKS[b], conv_eng=nc.vector)
            for t in range(KH):
                fc1_tgroup(b, t, hh[b])
        # w2 in-place: DVE takes the even tiles, Pool the early odd tiles,
        # ACT (free once the wave epilogues drain) the late odd tiles
        for t in range(KH):
            e = V if t % 2 == 0 else (nc.gpsimd if t < 12 else A)
            inplace_quant(w2q[t][:], e, "q2", D)
        first_fc2 = True
        while fc1_left or fc2_left:
            if fc2_left:
                b = fc2_left.pop(0)
                fc2_block(b, hh[b], split_out=not fc2_left,
                          t_order=T_PROD_ORDER if first_fc2 else None)
                first_fc2 = False
            if fc1_left:
                b = fc1_left.pop(0)
                xh[b] = load_x_block(starts[b], M_BLOCKS[b], conv_eng=nc.vector)
                for t in range(KH):
                    fc1_tgroup(b, t, hh[b])

    if walrus_fixups:
        _split_oversized_waits(nc)
    return nc


_PROGRAM_CACHE = {}


def _get_program(qmax: float, use_split: bool = False):
    key = qmax
    if key not in _PROGRAM_CACHE:
        _PROGRAM_CACHE[key] = build_program(qmax)
    return _PROGRAM_CACHE[key]


def kernel(x, w1, b1, w2, b2, bits):
    qmax = float(2.0 ** (int(bits) - 1) - 1.0)
    nc = _get_program(qmax)

    x = np.ascontiguousarray(np.asarray(x, dtype=np.float32)).reshape(M_TOTAL, D)
    w1t = np.ascontiguousarray(np.asarray(w1, dtype=np.float32).T)   # [768, 3072]
    w2t = np.ascontiguousarray(np.asarray(w2, dtype=np.float32).T)   # [3072, 768]
    b1h = np.ascontiguousarray(
        np.asarray(b1, dtype=np.float32).reshape(KH, 128).T
    )  # [128, KH]
    b2h = np.ascontiguousarray(
        np.asarray(b2, dtype=np.float32).reshape(KD, 128).T
    )  # [128, KD]
    xt_full = x.T                                                    # [768, 12544]

    ident = np.eye(128, dtype=np.float32)
    in_maps = []
    for c in range(N_CORES):
        xt_c = xt_full[:, c * M_SHARD:(c + 1) * M_SHARD]             # [768, 1568]
        # [768, M] -> [KD, 128, M] -> [128, KD, M]
        xt_c = np.ascontiguousarray(
            xt_c.reshape(KD, 128, M_SHARD).transpose(1, 0, 2)
        )
        in_maps.append(
            {"xt": xt_c, "w1t": w1t, "w2t": w2t, "b1": b1h, "b2": b2h,
             "ident": ident}
        )

    res = bass_utils.run_bass_kernel_spmd(nc, in_maps, core_ids=list(range(N_CORES)))
    outs = []
    for c in range(N_CORES):
        ot = res.results[c]["outT"]                                  # [128, KD, M]
        outs.append(ot.transpose(1, 0, 2).reshape(D, M_SHARD).T)     # [M, 768]
    out = np.concatenate(outs, axis=0)
    return np.ascontiguousarray(out.reshape(B, S, D))
